# revision 1
# baseline (speedup 1.0000x reference)
"""Trainium2 Bass kernel for EnhancedMaskLoss (CE + dice + BCE mask loss).

Math: the reference samples NP=45000 points per scene via sample_idx and
computes BCE/dice over matched (query, target) pairs.  All sampled sums are
rewritten as count-weighted sums over the full point dim:

    sum_j f(x[sample_idx[j]]) == sum_p count[p] * f(x[p])

so the device streams the full pred/target masks once and accumulates three
bf16 matmul passes per 128-point chunk into fp32 PSUM:

    O1[t, q] = sum_p (c*tgt^T)[p, t] * pred[p, q]          (BCE cross term)
    O2[t, q] = sum_p (c*tgt^T)[p, t] * sigmoid(pred)[p, q] (dice numer/Psum)
    O3[t, q] = sum_p (c*tgt^T)[p, t] * ln(1-sigmoid(pred)) (= -softplus sum)

plus a ones-moving column for Tsum.  The stationary carries an extra c-row
(-> per-query sums) and a zero pad row.  The tiny [32,~101] outputs are
combined on the host (gather of 30 matched columns + dice division +
weighting).  Phase B (Ln) is fenced behind phase A (Sigmoid) via a computed
bias tile so the ACT table is loaded exactly twice.

Sharding: 8 cores, scene b = core//2, each core takes half the points
(40064 = 313*128; halves overlap by 128 points, counts zeroed on one side).
Within a core, points are blocked into DMA groups of 32 chunks laid out so
every DMA moves >=512B-contiguous per-partition runs.  CE runs on even cores
with real logits, on odd cores with zeroed weights (exp via sigmoid to avoid
a third ACT table).
"""

import numpy as np

import concourse.bacc as bacc
import concourse.bass as bass
import concourse.mybir as mybir
import concourse.tile as tile

B, Q, M, P, NP = 4, 100, 30, 80000, 45000
NUM_CLASSES = 20
EOS_COEF = 0.1
W_CE, W_DICE, W_MASK = 2.0, 5.0, 5.0
NCLS = NUM_CLASSES + 1  # 21

SHARD = 40064          # points per core = 313 * 128
NCH = SHARD // 128     # 313 chunks
DG = 32                # chunks per DMA/activation group
TT = 32                # stationary width: 30 targets + c row + zero pad

f32 = mybir.dt.float32
bf16 = mybir.dt.bfloat16
AF = mybir.ActivationFunctionType


def _groups(nch):
    gs = []
    base = 0
    while base < nch:
        g = min(DG, nch - base)
        # avoid a tiny tail: balance the last two groups
        if 0 < nch - base - g < 8 and g == DG:
            g = (nch - base + 1) // 2
        gs.append((base, g))
        base += g
    return gs


def build_nc(nch=NCH):
    nc = bacc.Bacc(None, target_bir_lowering=False)

    groups = _groups(nch)
    ngroups = len(groups)

    pred_ds = []
    tgt_ds = []
    for gi, (base, gs) in enumerate(groups):
        pred_ds.append(nc.dram_tensor(
            f"pred{gi}", [128, gs * 100], bf16, kind="ExternalInput"))
        tgt_ds.append(nc.dram_tensor(
            f"tgt{gi}", [128, gs, TT], bf16, kind="ExternalInput"))
    c_d = nc.dram_tensor("cmat", [128, nch], f32, kind="ExternalInput")
    lg_d = nc.dram_tensor("logits", [Q, NCLS], f32, kind="ExternalInput")
    wo_d = nc.dram_tensor("wo", [Q, 2], f32, kind="ExternalInput")
    w1h_d = nc.dram_tensor("w1h", [Q, NCLS], f32, kind="ExternalInput")

    o1_d = nc.dram_tensor("o1", [TT, Q], f32, kind="ExternalOutput")
    o4_d = nc.dram_tensor("o4", [TT, 1], f32, kind="ExternalOutput")
    o2_d = nc.dram_tensor("o2", [TT, Q], f32, kind="ExternalOutput")
    o3_d = nc.dram_tensor("o3", [TT, Q], f32, kind="ExternalOutput")
    oce_d = nc.dram_tensor("oce", [2, 2], f32, kind="ExternalOutput")

    with tile.TileContext(nc) as tc:
        with (
            tc.tile_pool(name="const", bufs=1) as constp,
            tc.tile_pool(name="io", bufs=4) as iop,
            tc.tile_pool(name="sres", bufs=ngroups) as sresp,
            tc.tile_pool(name="tcres", bufs=ngroups) as tcresp,
            tc.tile_pool(name="lpool", bufs=4) as lp,
            tc.tile_pool(name="psum", bufs=1, space="PSUM") as psump,
        ):
            # First pred/tgt DMAs go first so ACT's sigmoid stream starts ASAP
            pred_tiles = {}
            tgt_tiles = {}
            for gi in range(min(2, ngroups)):
                bg, gg = groups[gi]
                pt = iop.tile([128, gg * 100], bf16, tag="pred")
                if gi == 0:
                    for s0 in range(0, gg, 8):
                        ss = min(8, gg - s0)
                        nc.sync.dma_start(
                            pt[:, 100 * s0 : 100 * (s0 + ss)],
                            pred_ds[gi][:, 100 * s0 : 100 * (s0 + ss)])
                else:
                    nc.sync.dma_start(pt[:, :], pred_ds[gi][:, :])
                tt_ = iop.tile([128, gg, TT], bf16, tag="tgt")
                nc.sync.dma_start(tt_[:, :, :], tgt_ds[gi][:, :, :])
                pred_tiles[gi] = pt
                tgt_tiles[gi] = tt_

            c_all = constp.tile([128, nch], f32, tag="call")
            nc.gpsimd.dma_start(c_all[:, :], c_d[:, :])
            ones_t = constp.tile([128, 1], bf16, tag="ones")
            nc.gpsimd.memset(ones_t[:, :], 1.0)

            o1_ps = psump.tile([TT, Q], f32, tag="o1")
            o4_ps = psump.tile([TT, 1], f32, tag="o4")
            o2_ps = psump.tile([TT, Q], f32, tag="o2")
            o3_ps = psump.tile([TT, Q], f32, tag="o3")
            oce_ps = psump.tile([2, 2], f32, tag="oce")

            # ---- phase A: stream pred/tgt, raw + sigmoid matmul passes ----
            s_tiles = []
            tc_tiles = []
            for gi, (base, gs) in enumerate(groups):
                if gi in pred_tiles:
                    pred_t = pred_tiles[gi]
                    tgt_t = tgt_tiles[gi]
                else:
                    pred_t = iop.tile([128, gs * 100], bf16, tag="pred")
                    nc.sync.dma_start(pred_t[:, :], pred_ds[gi][:, :])
                    tgt_t = iop.tile([128, gs, TT], bf16, tag="tgt")
                    nc.sync.dma_start(tgt_t[:, :, :], tgt_ds[gi][:, :, :])

                tc_t = tcresp.tile([128, gs, TT], bf16, tag="tc")
                for j in range(gs):
                    eng = nc.vector if j % 2 == 0 else nc.gpsimd
                    eng.tensor_scalar_mul(
                        tc_t[:, j, :], tgt_t[:, j, :],
                        c_all[:, base + j : base + j + 1],
                    )

                s_t = sresp.tile([128, gs * 100], bf16, tag="s")
                if gi == 0:
                    for s0 in range(0, gs, 8):
                        ss = min(8, gs - s0)
                        nc.scalar.activation(
                            s_t[:, 100 * s0 : 100 * (s0 + ss)],
                            pred_t[:, 100 * s0 : 100 * (s0 + ss)], AF.Sigmoid)
                else:
                    nc.scalar.activation(s_t[:, :], pred_t[:, :], AF.Sigmoid)

                for j in range(gs):
                    kk = base + j
                    st_f = kk == 0
                    sp_f = kk == nch - 1
                    sl = slice(100 * j, 100 * (j + 1))
                    nc.tensor.matmul(
                        o1_ps[:, :], tc_t[:, j, :], pred_t[:, sl],
                        start=st_f, stop=sp_f)
                    nc.tensor.matmul(
                        o4_ps[:, :], tc_t[:, j, :], ones_t[:, :],
                        start=st_f, stop=sp_f)
                    nc.tensor.matmul(
                        o2_ps[:, :], tc_t[:, j, :], s_t[:, sl],
                        start=st_f, stop=sp_f)
                s_tiles.append(s_t)
                tc_tiles.append(tc_t)

            # ---- CE, sigmoid-table part: exp(x) = s/(1-s), s = sigmoid(x)
            lg_t = constp.tile([Q, NCLS], f32, tag="lg")
            nc.gpsimd.dma_start(lg_t[:, :], lg_d[:, :])
            w1h_t = constp.tile([Q, NCLS], f32, tag="w1h")
            nc.gpsimd.dma_start(w1h_t[:, :], w1h_d[:, :])
            wo_t = constp.tile([Q, 2], f32, tag="wo")
            nc.gpsimd.dma_start(wo_t[:, :], wo_d[:, :])

            ce_zb = constp.tile([128, 1], f32, tag="ce_zb")
            nc.vector.tensor_scalar(
                ce_zb[:, :], s_tiles[0][:, 0:1], 0.0, 0.0,
                mybir.AluOpType.mult, mybir.AluOpType.add,
            )
            slg_t = constp.tile([Q, NCLS], f32, tag="ce_slg")
            nc.scalar.activation(
                slg_t[:, :], lg_t[:, :], AF.Sigmoid, bias=ce_zb[0:Q, :])
            om_t = constp.tile([Q, NCLS], f32, tag="ce_om")
            nc.vector.tensor_scalar(
                om_t[:, :], slg_t[:, :], -1.0, 1.0,
                mybir.AluOpType.mult, mybir.AluOpType.add,
            )
            rec_t = constp.tile([Q, NCLS], f32, tag="ce_rec")
            nc.vector.reciprocal(rec_t[:, :], om_t[:, :])
            ex_t = constp.tile([Q, NCLS], f32, tag="ce_ex")
            nc.vector.tensor_tensor(
                ex_t[:, :], slg_t[:, :], rec_t[:, :], mybir.AluOpType.mult
            )
            se_t = constp.tile([Q, 1], f32, tag="ce_se")
            nc.vector.tensor_reduce(
                se_t[:, :], ex_t[:, :], mybir.AxisListType.X, mybir.AluOpType.add
            )
            rl_t = constp.tile([Q, 2], f32, tag="ce_rl")
            z2_t = constp.tile([Q, NCLS], f32, tag="ce_z2")
            nc.vector.tensor_tensor(
                z2_t[:, :], lg_t[:, :], w1h_t[:, :], mybir.AluOpType.mult
            )
            nc.vector.tensor_reduce(
                rl_t[:, 1:2], z2_t[:, :], mybir.AxisListType.X, mybir.AluOpType.add
            )

            # o1/o4/o2 accumulations are complete: drain them during phase B
            for ps, dram, w in ((o1_ps, o1_d, Q), (o2_ps, o2_d, Q),
                                (o4_ps, o4_d, 1)):
                sb = iop.tile([TT, w], f32, tag="osb")
                nc.vector.tensor_copy(sb[:, :], ps[:, :])
                nc.sync.dma_start(dram[:, :], sb[:, :])

            # Barrier: phase-B Ln reads a bias computed from the last sigmoid
            # output so the scheduler cannot interleave Ln into the sigmoid
            # stream (ACT table thrash).
            dep_src = s_tiles[-1][:, 0:1]
            one_bias = constp.tile([128, 1], f32, tag="one_bias")
            nc.scalar.activation(
                one_bias[:, :], dep_src, AF.Copy, bias=1.0, scale=0.0
            )
            zero_bias = constp.tile([128, 1], f32, tag="zero_bias")
            nc.vector.tensor_scalar(
                zero_bias[:, :], dep_src, 0.0, 0.0,
                mybir.AluOpType.mult, mybir.AluOpType.add,
            )

            # ---- phase B: ln(1 - s) pass ----
            for gi, (base, gs) in enumerate(groups):
                sub = 8 if gi == ngroups - 1 else gs
                for s0 in range(0, gs, sub):
                    ss = min(sub, gs - s0)
                    l_t = lp.tile([128, ss * 100], bf16, tag="l")
                    nc.scalar.activation(
                        l_t[:, :],
                        s_tiles[gi][:, 100 * s0 : 100 * (s0 + ss)], AF.Ln,
                        bias=one_bias[:, :], scale=-1.0,
                    )
                    for j in range(ss):
                        kk = base + s0 + j
                        nc.tensor.matmul(
                            o3_ps[:, :], tc_tiles[gi][:, s0 + j, :],
                            l_t[:, 100 * j : 100 * (j + 1)],
                            start=(kk == 0), stop=(kk == nch - 1))
                if gi == 0:
                    # CE Ln + oce drain early in phase B (same ACT table)
                    nc.scalar.activation(
                        rl_t[:, 0:1], se_t[:, :], AF.Ln, bias=zero_bias[0:Q, :]
                    )
                    nc.tensor.matmul(oce_ps[:, :], wo_t[:, :], rl_t[:, :])
                    oce_sb0 = iop.tile([2, 2], f32, tag="ocesb")
                    nc.vector.tensor_copy(oce_sb0[:, :], oce_ps[:, :])
                    nc.sync.dma_start(oce_d[:, :], oce_sb0[:, :])


            # ---- outputs ----
            sb3 = iop.tile([TT, Q], f32, tag="osb")
            nc.vector.tensor_copy(sb3[:, :], o3_ps[:, :])
            nc.sync.dma_start(o3_d[:, :], sb3[:, :])

    nc.compile()
    return nc


def _interleave_views(flat, groups):
    """flat: [shard, W] row-major. Returns per-group [128, gs*W] (or
    [128, gs, W]) arrays with point(g, p, j) = base*128 + gs*p + j."""
    outs = []
    for base, gs in groups:
        blk = flat[base * 128 : (base + gs) * 128]
        outs.append(np.ascontiguousarray(blk.reshape(128, gs * blk.shape[1])))
    return outs


def host_prep(pred_logits, pred_masks, target_masks, target_classes,
              src_idx, tgt_idx, sample_idx, nch=NCH):
    """Build per-core input maps + aux data for the final combine."""
    shard = nch * 128
    groups = _groups(nch)
    npbf = mybir.dt.np(bf16)
    cls_w = np.ones(NCLS, np.float32)
    cls_w[0] = 0.0
    cls_w[-1] = EOS_COEF

    in_maps = []
    aux = {"gidx": [], "wsum": 0.0, "groups": groups}
    for b in range(B):
        inv = np.argsort(tgt_idx[b])
        aux["gidx"].append(src_idx[b][inv].copy())

        tc_full = np.full(Q, NUM_CLASSES, np.int64)
        tc_full[src_idx[b]] = target_classes[b][tgt_idx[b]]
        wq = cls_w[tc_full]
        aux["wsum"] += float(wq.sum())
        w1h = wq[:, None] * np.eye(NCLS, dtype=np.float32)[tc_full]
        wo = np.stack([wq, np.ones(Q, np.float32)], axis=1)

        c_full = np.bincount(sample_idx[b], minlength=P).astype(np.float32)

        for h in range(2):
            off = 0 if h == 0 else P - shard
            pred_sh = pred_masks[b, off : off + shard, :].astype(npbf)
            # point-major target with c-row of ones and zero pad
            tpf = np.zeros((shard, TT), npbf)
            tpf[:, :M] = target_masks[b][:, off : off + shard].T
            tpf[:, M] = 1.0
            c_sh = c_full[off : off + shard].copy()
            if h == 1:
                c_sh[: 2 * shard - P] = 0.0  # overlap owned by core h=0

            im = {
                "logits": pred_logits[b] if h == 0 else np.zeros((Q, NCLS), np.float32),
                "wo": wo if h == 0 else np.zeros((Q, 2), np.float32),
                "w1h": w1h if h == 0 else np.zeros((Q, NCLS), np.float32),
            }
            cmat = np.empty((128, nch), np.float32)
            for gi, (base, gs) in enumerate(groups):
                blk = slice(base * 128, (base + gs) * 128)
                im[f"pred{gi}"] = np.ascontiguousarray(
                    pred_sh[blk].reshape(128, gs * 100))
                im[f"tgt{gi}"] = np.ascontiguousarray(
                    tpf[blk].reshape(128, gs, TT))
                cmat[:, base : base + gs] = c_sh[blk].reshape(128, gs)
            im["cmat"] = cmat
            in_maps.append(im)
    return in_maps, aux


def host_combine(results, aux):
    """results: list of 8 dicts with o1/o2/o3/oce. Returns [3] f32."""
    bce_total = 0.0
    dice_total = 0.0
    ce_num = 0.0
    idx30 = np.arange(M)
    for b in range(B):
        gidx = aux["gidx"][b]
        r0, r1 = results[2 * b], results[2 * b + 1]
        O1 = r0["o1"].astype(np.float64) + r1["o1"]
        O2 = r0["o2"].astype(np.float64) + r1["o2"]
        O3 = r0["o3"].astype(np.float64) + r1["o3"]
        ce_num += (r0["oce"][0, 0] - r0["oce"][1, 1])
        ce_num += (r1["oce"][0, 0] - r1["oce"][1, 1])

        O4 = r0["o4"].astype(np.float64) + r1["o4"]
        X1 = O1[idx30, gidx].sum()
        Tsum = O4[idx30, 0]
        Num = O2[idx30, gidx]
        Psum = O2[M, gidx]
        Abce = -(O3[M, gidx].sum())
        bce_total += Abce - X1
        dice_total += (1.0 - (2.0 * Num + 1.0) / (Psum + Tsum + 1.0)).sum()

    num_masks = B * M
    loss_ce = ce_num / max(aux["wsum"], 1e-8)
    loss_mask = bce_total / NP / num_masks
    loss_dice = dice_total / num_masks
    return np.array([W_CE * loss_ce, W_DICE * loss_dice, W_MASK * loss_mask],
                    np.float32)


_NC_CACHE = {}


def kernel(pred_logits, pred_masks, target_masks, target_classes,
           src_idx, tgt_idx, sample_idx):
    from concourse.bass_utils import run_bass_kernel_spmd

    pred_logits = np.asarray(pred_logits, np.float32)
    pred_masks = np.asarray(pred_masks, np.float32)
    target_masks = np.asarray(target_masks, np.float32)
    target_classes = np.asarray(target_classes)
    src_idx = np.asarray(src_idx)
    tgt_idx = np.asarray(tgt_idx)
    sample_idx = np.asarray(sample_idx)

    if "nc" not in _NC_CACHE:
        _NC_CACHE["nc"] = build_nc()
    nc = _NC_CACHE["nc"]
    in_maps, aux = host_prep(
        pred_logits, pred_masks, target_masks, target_classes,
        src_idx, tgt_idx, sample_idx)
    res = run_bass_kernel_spmd(nc, in_maps, core_ids=list(range(8)))
    return host_combine(res.results, aux)



# revision 34
# speedup vs baseline: 3.9143x; 3.9143x over previous
"""Trainium2 Bass kernel for EnhancedMaskLoss (CE + dice + BCE mask loss).

Math: the reference samples NP=45000 points per scene via sample_idx and
computes BCE/dice over matched (query, target) pairs.  All sampled sums are
count-weighted sums over distinct points:

    sum_j f(x[sample_idx[j]]) == sum_p count[p] * f(x[p])

Only points with count>0 contribute (~34.5K of 80000 per scene), and only the
M=30 matched query columns of pred_masks enter the loss, so the host packs a
compressed [ncomp, 61] view per scene (x | c*t | c -- pure gather/layout).
Each of the 8 cores takes half a scene's compressed points, chunked into
128-point partitions.

Per chunk the device computes e=exp(x) and sp=ln(1+e)=softplus on ACT (both
live in the natural_log_exp table, one load), z=1+e and r=1/z=1-sigmoid on
DVE, and accumulates two narrow matmuls against the 31-wide moving [c*t | c]:

    OA[0:30]  = x^T  @ [c*t | c]   -> diag = sum(c*t*x)      (BCE cross term)
    OA[30:60] = r^T  @ [c*t | c]   -> diag/c-col give sigmoid sums via
                                      sum(c*t*s) = Tsum - sum(c*t*r),
                                      sum(c*s)   = NP   - sum(c*r)
    OA[60:90] = sp^T @ [c*t | c]   -> c-col = sum(c*softplus) (BCE term)

sum(c*t) (dice target-sum) depends only on inputs and is summed on host.
CE runs on even cores: exp/ln on ACT (same table), weighted sums via one tiny
matmul.
"""

import numpy as np

import concourse.bacc as bacc
import concourse.bass as bass
import concourse.mybir as mybir
import concourse.tile as tile

B, Q, M, P, NP = 4, 100, 30, 80000, 45000
NUM_CLASSES = 20
EOS_COEF = 0.1
W_CE, W_DICE, W_MASK = 2.0, 5.0, 5.0
NCLS = NUM_CLASSES + 1  # 21

NCH = 138              # 128-point chunks per core (capacity 17664 points)
SH = NCH * 128
XSLICES = [12, 12, 20, 32, 32, 24, 6]  # x DMA slices (chunks)
TCSLICES = [9, 35, 32, 32, 30]         # c*t|c DMA slices (Pool queue)
ASPANS = [12, 32, 64, 24, 6]           # ACT/DVE pass sizes (chunks)
LNEXP_TABLE = "natural_log_exp_and_others"

f32 = mybir.dt.float32
bf16 = mybir.dt.bfloat16
fp8 = mybir.dt.float8e4
AF = mybir.ActivationFunctionType
ALU = mybir.AluOpType


def _spans(sizes, nch=NCH):
    gs, base = [], 0
    for g in sizes:
        gs.append((base, g))
        base += g
    assert base == nch
    return gs


class _Bacc(bacc.Bacc):
    """Bacc whose act-table-load placement sees Exp/Ln only in the shared
    natural_log_exp table, so the greedy pass emits a single load of the
    real (hardware-identical) combined table instead of ping-ponging
    between the exp-only and ln-only tables."""

    def insert_act_table_loads(self):
        import bass_rust as _bass_rust
        from concourse.hw_specs import get_activation_tables

        has_activation = any(
            isinstance(i, mybir.InstActivation)
            for b in self.main_func.blocks
            for i in b.instructions
        )
        if not has_activation:
            return
        exp_ln = {mybir.ActivationFunctionType.Exp,
                  mybir.ActivationFunctionType.Ln}
        tables = []
        for name, funcs in get_activation_tables(self.m.arch).items():
            if name != LNEXP_TABLE:
                funcs = funcs - exp_ln
            tables.append((name, funcs))
        assert any(exp_ln <= set(f) for _, f in tables), "no exp+ln table"
        _bass_rust.insert_act_table_loads(self, tables)


def build_nc(nch=NCH):
    nc = _Bacc(None, target_bir_lowering=False)

    xslices = _spans(XSLICES, nch)
    tcslices = _spans(TCSLICES, nch)
    aspans = _spans(ASPANS, nch)

    x_d = nc.dram_tensor("xin", [128, nch, M], bf16, kind="ExternalInput")
    tc_d = nc.dram_tensor("tcx", [128, nch, M + 1], fp8, kind="ExternalInput")
    # CE consts merged: cols 0:21 logits | 21:42 w1h | 42:44 wo
    ce_d = nc.dram_tensor("ce", [Q, 2 * NCLS + 2], f32, kind="ExternalInput")

    oa_d = nc.dram_tensor("oa", [64 + 2 * M, M + 1], f32, kind="ExternalOutput")

    with tile.TileContext(nc) as tc_:
        with (
            tc_.tile_pool(name="const", bufs=1) as constp,
            tc_.tile_pool(name="ez", bufs=3) as ezp,
            tc_.tile_pool(name="io", bufs=2) as iop,
            tc_.tile_pool(name="psum", bufs=1, space="PSUM") as psump,
        ):
            o1_ps = psump.tile([M, M + 1], f32, tag="o1")
            o2_ps = psump.tile([2 * M, M + 1], f32, tag="o2")
            oce_ps = psump.tile([2, 2], f32, tag="oce")

            x_t = constp.tile([128, nch, M], bf16, tag="x")
            tc_t = constp.tile([128, nch, M + 1], fp8, tag="tc")
            z_t = constp.tile([128, nch, M], bf16, tag="z")
            rs_t = constp.tile([128, nch, 2 * M], bf16, tag="rs")

            one_t = constp.tile([128, 1], f32, tag="one")
            nc.gpsimd.memset(one_t[:, :], 1.0)

            ce_t = constp.tile([Q, 2 * NCLS + 2], f32, tag="ce")
            nc.gpsimd.dma_start(ce_t[:, :], ce_d[:, :])
            # c*t|c slices ride the Pool SWDGE queue so x owns HWDGE
            for bt, gt in tcslices:
                tl = slice(bt, bt + gt)
                nc.gpsimd.dma_start(tc_t[:, tl, :], tc_d[:, tl, :])

            # x DMA slices; x-block matmuls trail their slices.
            for base, g in xslices:
                sl = slice(base, base + g)
                nc.sync.dma_start(x_t[:, sl, :], x_d[:, sl, :])
                for j in range(base, base + g):
                    nc.tensor.matmul(
                        o1_ps[:, :], x_t[:, j, :], tc_t[:, j, :],
                        start=(j == 0), stop=(j == nch - 1))

            for si, (base, g) in enumerate(aspans):
                sl = slice(base, base + g)
                # ACT: e = exp(x), softplus = ln(1 + e) via the bias input;
                # the exp->ln chain stays entirely on ACT (one table).
                e_t = ezp.tile([128, g, M], bf16, tag="e")
                nc.scalar.activation(e_t[:, :, :], x_t[:, sl, :], AF.Exp)
                nc.scalar.activation(rs_t[:, sl, M:2 * M], e_t[:, :, :],
                                     AF.Ln, bias=one_t[:, :])
                # DVE: z = 1 + e, r = 1/z (= 1 - sigmoid); feeds only the
                # o2 matmuls, off the ACT critical path.
                nc.vector.tensor_scalar_add(z_t[:, sl, :], e_t[:, :, :], 1.0)
                with nc.allow_low_precision(reason="r in [0,1], bf16 ok"):
                    nc.vector.reciprocal(rs_t[:, sl, 0:M], z_t[:, sl, :])
                for j in range(base, base + g):
                    nc.tensor.matmul(
                        o2_ps[:, :], rs_t[:, j, :], tc_t[:, j, :],
                        start=(j == 0), stop=(j == nch - 1))

            # Drains on ACT (free at the end; PSUM-capable); oce rides in
            # the padding rows of the single oa output.
            oa_sb = iop.tile([64 + 2 * M, M + 1], f32, tag="oasb")
            nc.scalar.activation(oa_sb[0:M, :], o1_ps[:, :], AF.Copy)
            nc.scalar.activation(oa_sb[32:34, 0:2], oce_ps[:, :], AF.Copy)
            nc.scalar.activation(oa_sb[64:64 + 2 * M, :], o2_ps[:, :],
                                 AF.Copy)
            nc.sync.dma_start(oa_d[:, :], oa_sb[:, :])

    nc.compile()
    return nc


def host_prep(pred_logits, pred_masks, target_masks, target_classes,
              src_idx, tgt_idx, sample_idx, nch=NCH):
    """Compress/gather per-scene inputs and build per-core input maps."""
    sh = nch * 128
    npbf = mybir.dt.np(bf16)
    npf8 = mybir.dt.np(fp8)
    cls_w = np.ones(NCLS, np.float32)
    cls_w[0] = 0.0
    cls_w[-1] = EOS_COEF

    in_maps = []
    aux = {"tsum": [], "wsum": 0.0}
    zero_ce = np.zeros((Q, 2 * NCLS + 2), np.float32)
    for b in range(B):
        inv = np.argsort(tgt_idx[b])
        gidx = src_idx[b][inv]

        tc_full = np.full(Q, NUM_CLASSES, np.int64)
        tc_full[src_idx[b]] = target_classes[b][tgt_idx[b]]
        wq = cls_w[tc_full]
        aux["wsum"] += float(wq.sum())
        ce = np.zeros((Q, 2 * NCLS + 2), np.float32)
        ce[:, 0:NCLS] = pred_logits[b]
        ce[:, NCLS:2 * NCLS] = wq[:, None] * np.eye(NCLS, dtype=np.float32)[tc_full]
        ce[:, 2 * NCLS] = wq
        ce[:, 2 * NCLS + 1] = 1.0

        c_full = np.bincount(sample_idx[b], minlength=P)
        nz = np.flatnonzero(c_full)
        cnz = c_full[nz].astype(np.float64)
        ncomp = len(nz)
        assert ncomp <= 2 * sh, f"compressed points {ncomp} exceed capacity"

        assert cnz.max() <= 16, "counts exceed fp8-exact range"
        packed = np.zeros((2 * sh, 2 * M + 1), np.float32)
        packed[:ncomp, 0:M] = pred_masks[b][nz][:, gidx]
        ct = target_masks[b][:, nz].T * cnz[:, None]
        packed[:ncomp, M:2 * M] = ct
        packed[:ncomp, 2 * M] = cnz
        aux["tsum"].append(ct.sum(0))

        n0 = (ncomp + 1) // 2
        # half 1 starts at a fresh offset so each core sees sh points
        half1 = np.zeros((sh, 2 * M + 1), np.float32)
        half1[:ncomp - n0] = packed[n0:ncomp]
        for h, arr in enumerate((packed[:sh], half1)):
            a128 = np.ascontiguousarray(
                arr.reshape(nch, 128, 2 * M + 1).transpose(1, 0, 2))
            in_maps.append({
                "xin": np.ascontiguousarray(a128[:, :, 0:M]).astype(npbf),
                "tcx": np.ascontiguousarray(
                    a128[:, :, M:2 * M + 1]).astype(npf8),
                "ce": ce if h == 0 else zero_ce,
            })
    return in_maps, aux


def host_combine(results, aux):
    """results: list of 8 dicts with oa/oce. Returns [3] f32."""
    bce_total = 0.0
    dice_total = 0.0
    ce_num = 0.0
    idx = np.arange(M)
    for b in range(B):
        r0, r1 = results[2 * b], results[2 * b + 1]
        OA = r0["oa"].astype(np.float64) + r1["oa"]
        ce_num += float(r0["oa"][32, 0] - r0["oa"][33, 1])
        ce_num += float(r1["oa"][32, 0] - r1["oa"][33, 1])

        Tsum = aux["tsum"][b]
        X1 = OA[idx, idx].sum()
        ScS = Tsum - OA[64 + idx, idx]     # sum c*t*sigmoid
        Psum = NP - OA[64 + idx, M]        # sum c*sigmoid
        A = OA[64 + M + idx, M]            # sum c*softplus

        bce_total += A.sum() - X1
        dice_total += (1.0 - (2.0 * ScS + 1.0) / (Psum + Tsum + 1.0)).sum()

    num_masks = B * M
    loss_ce = ce_num / max(aux["wsum"], 1e-8)
    loss_mask = bce_total / NP / num_masks
    loss_dice = dice_total / num_masks
    return np.array([W_CE * loss_ce, W_DICE * loss_dice, W_MASK * loss_mask],
                    np.float32)


_NC_CACHE = {}


def kernel(pred_logits, pred_masks, target_masks, target_classes,
           src_idx, tgt_idx, sample_idx):
    from concourse.bass_utils import run_bass_kernel_spmd

    pred_logits = np.asarray(pred_logits, np.float32)
    pred_masks = np.asarray(pred_masks, np.float32)
    target_masks = np.asarray(target_masks, np.float32)
    target_classes = np.asarray(target_classes)
    src_idx = np.asarray(src_idx)
    tgt_idx = np.asarray(tgt_idx)
    sample_idx = np.asarray(sample_idx)

    if "nc" not in _NC_CACHE:
        _NC_CACHE["nc"] = build_nc()
    nc = _NC_CACHE["nc"]
    in_maps, aux = host_prep(
        pred_logits, pred_masks, target_masks, target_classes,
        src_idx, tgt_idx, sample_idx)
    res = run_bass_kernel_spmd(nc, in_maps, core_ids=list(range(8)))
    return host_combine(res.results, aux)


# revision 37
# speedup vs baseline: 3.9505x; 1.0092x over previous
"""Trainium2 Bass kernel for EnhancedMaskLoss (CE + dice + BCE mask loss).

Math: the reference samples NP=45000 points per scene via sample_idx and
computes BCE/dice over matched (query, target) pairs.  All sampled sums are
count-weighted sums over distinct points:

    sum_j f(x[sample_idx[j]]) == sum_p count[p] * f(x[p])

Only points with count>0 contribute (~34.5K of 80000 per scene), and only the
M=30 matched query columns of pred_masks enter the loss, so the host packs a
compressed [ncomp, 61] view per scene (x | c*t | c -- pure gather/layout).
Each of the 8 cores takes half a scene's compressed points, chunked into
128-point partitions.

Per chunk the device computes e=exp(x) and sp=ln(1+e)=softplus on ACT (both
live in the natural_log_exp table, one load), z=1+e and r=1/z=1-sigmoid on
DVE, and accumulates two narrow matmuls against the 31-wide moving [c*t | c]:

    OA[0:30]  = x^T  @ [c*t | c]   -> diag = sum(c*t*x)      (BCE cross term)
    OA[30:60] = r^T  @ [c*t | c]   -> diag/c-col give sigmoid sums via
                                      sum(c*t*s) = Tsum - sum(c*t*r),
                                      sum(c*s)   = NP   - sum(c*r)
    OA[60:90] = sp^T @ [c*t | c]   -> c-col = sum(c*softplus) (BCE term)

sum(c*t) (dice target-sum) depends only on inputs and is summed on host.
CE runs on even cores: exp/ln on ACT (same table), weighted sums via one tiny
matmul.
"""

import numpy as np

import concourse.bacc as bacc
import concourse.bass as bass
import concourse.mybir as mybir
import concourse.tile as tile

B, Q, M, P, NP = 4, 100, 30, 80000, 45000
NUM_CLASSES = 20
EOS_COEF = 0.1
W_CE, W_DICE, W_MASK = 2.0, 5.0, 5.0
NCLS = NUM_CLASSES + 1  # 21

NCH = 138              # 128-point chunks per core (capacity 17664 points)
SH = NCH * 128
XSLICES = [12, 12, 20, 32, 32, 24, 6]  # x DMA slices (chunks)
TCSLICES = [9, 35, 32, 32, 30]         # c*t|c DMA slices (Pool queue)
ASPANS = [12, 32, 64, 24, 6]           # ACT/DVE pass sizes (chunks)
LNEXP_TABLE = "natural_log_exp_and_others"

f32 = mybir.dt.float32
bf16 = mybir.dt.bfloat16
fp8 = mybir.dt.float8e4
AF = mybir.ActivationFunctionType
ALU = mybir.AluOpType


def _spans(sizes, nch=NCH):
    gs, base = [], 0
    for g in sizes:
        gs.append((base, g))
        base += g
    assert base == nch
    return gs


class _Bacc(bacc.Bacc):
    """Bacc whose act-table-load placement sees Exp/Ln only in the shared
    natural_log_exp table, so the greedy pass emits a single load of the
    real (hardware-identical) combined table instead of ping-ponging
    between the exp-only and ln-only tables."""

    def insert_act_table_loads(self):
        import bass_rust as _bass_rust
        from concourse.hw_specs import get_activation_tables

        has_activation = any(
            isinstance(i, mybir.InstActivation)
            for b in self.main_func.blocks
            for i in b.instructions
        )
        if not has_activation:
            return
        exp_ln = {mybir.ActivationFunctionType.Exp,
                  mybir.ActivationFunctionType.Ln}
        tables = []
        for name, funcs in get_activation_tables(self.m.arch).items():
            if name != LNEXP_TABLE:
                funcs = funcs - exp_ln
            tables.append((name, funcs))
        assert any(exp_ln <= set(f) for _, f in tables), "no exp+ln table"
        _bass_rust.insert_act_table_loads(self, tables)


def build_nc(nch=NCH):
    nc = _Bacc(None, target_bir_lowering=False)

    xslices = _spans(XSLICES, nch)
    tcslices = _spans(TCSLICES, nch)
    aspans = _spans(ASPANS, nch)

    x_d = nc.dram_tensor("xin", [128, nch, M], bf16, kind="ExternalInput")
    tc_d = nc.dram_tensor("tcx", [128, nch, M + 1], fp8, kind="ExternalInput")
    # CE consts merged: cols 0:21 logits | 21:42 w1h | 42:44 wo
    ce_d = nc.dram_tensor("ce", [Q, 2 * NCLS + 2], f32, kind="ExternalInput")

    oa_d = nc.dram_tensor("oa", [64 + 2 * M, M + 1], f32, kind="ExternalOutput")

    with tile.TileContext(nc) as tc_:
        with (
            tc_.tile_pool(name="const", bufs=1) as constp,
            tc_.tile_pool(name="ez", bufs=3) as ezp,
            tc_.tile_pool(name="io", bufs=2) as iop,
            tc_.tile_pool(name="psum", bufs=1, space="PSUM") as psump,
        ):
            o1_ps = psump.tile([M, M + 1], f32, tag="o1")
            o2_ps = psump.tile([2 * M, M + 1], f32, tag="o2")
            oce_ps = psump.tile([2, 2], f32, tag="oce")

            x_t = constp.tile([128, nch, M], bf16, tag="x")
            tc_t = constp.tile([128, nch, M + 1], fp8, tag="tc")
            z_t = constp.tile([128, nch, M], bf16, tag="z")
            rs_t = constp.tile([128, nch, 2 * M], bf16, tag="rs")

            one_t = constp.tile([128, 1], f32, tag="one")
            nc.gpsimd.memset(one_t[:, :], 1.0)

            ce_t = constp.tile([Q, 2 * NCLS + 2], f32, tag="ce")
            nc.gpsimd.dma_start(ce_t[:, :], ce_d[:, :])
            # delay tc issues so early x transfers own the DMA engines
            delay_t = constp.tile([128, 2000], bf16, tag="delay")
            nc.gpsimd.memset(delay_t[:, :], 0.0)
            # c*t|c slices ride the Pool SWDGE queue so x owns HWDGE
            for bt, gt in tcslices:
                tl = slice(bt, bt + gt)
                nc.gpsimd.dma_start(tc_t[:, tl, :], tc_d[:, tl, :])

            # x DMA slices; x-block matmuls trail their slices.
            for base, g in xslices:
                sl = slice(base, base + g)
                nc.sync.dma_start(x_t[:, sl, :], x_d[:, sl, :])
                for j in range(base, base + g):
                    nc.tensor.matmul(
                        o1_ps[:, :], x_t[:, j, :], tc_t[:, j, :],
                        start=(j == 0), stop=(j == nch - 1))

            for si, (base, g) in enumerate(aspans):
                sl = slice(base, base + g)
                # ACT: e = exp(x), softplus = ln(1 + e) via the bias input;
                # the exp->ln chain stays entirely on ACT (one table).
                e_t = ezp.tile([128, g, M], bf16, tag="e")
                nc.scalar.activation(e_t[:, :, :], x_t[:, sl, :], AF.Exp)
                nc.scalar.activation(rs_t[:, sl, M:2 * M], e_t[:, :, :],
                                     AF.Ln, bias=one_t[:, :])
                # DVE: z = 1 + e, r = 1/z (= 1 - sigmoid); feeds only the
                # o2 matmuls, off the ACT critical path.
                nc.vector.tensor_scalar_add(z_t[:, sl, :], e_t[:, :, :], 1.0)
                with nc.allow_low_precision(reason="r in [0,1], bf16 ok"):
                    nc.vector.reciprocal(rs_t[:, sl, 0:M], z_t[:, sl, :])
                for j in range(base, base + g):
                    nc.tensor.matmul(
                        o2_ps[:, :], rs_t[:, j, :], tc_t[:, j, :],
                        start=(j == 0), stop=(j == nch - 1))

                if si == 0:
                    # CE compute fills the early DMA-wait stall on ACT;
                    # each engine's chain is self-contained (accumulator
                    # outputs), and nothing consumes oce until the drain.
                    elg = constp.tile([Q, NCLS], f32, tag="elg")
                    rl_t = constp.tile([Q, 2], f32, tag="rl")
                    se_t = constp.tile([Q, 1], f32, tag="se")
                    nc.scalar.activation(elg[:, :], ce_t[:, 0:NCLS], AF.Exp,
                                         accum_out=se_t[:, :])
                    nc.scalar.activation(rl_t[:, 0:1], se_t[:, :], AF.Ln)
                    wl_t = constp.tile([Q, NCLS], f32, tag="wl")
                    nc.vector.scalar_tensor_tensor(
                        wl_t[:, :], ce_t[:, 0:NCLS], 1.0,
                        ce_t[:, NCLS:2 * NCLS], ALU.mult, ALU.mult,
                        accum_out=rl_t[:, 1:2])
                    nc.tensor.matmul(
                        oce_ps[:, :], ce_t[:, 2 * NCLS:2 * NCLS + 2],
                        rl_t[:, :])

            # Drains on ACT (free at the end; PSUM-capable); oce rides in
            # the padding rows of the single oa output.
            oa_sb = iop.tile([64 + 2 * M, M + 1], f32, tag="oasb")
            nc.scalar.activation(oa_sb[0:M, :], o1_ps[:, :], AF.Copy)
            nc.scalar.activation(oa_sb[32:34, 0:2], oce_ps[:, :], AF.Copy)
            nc.scalar.activation(oa_sb[64:64 + 2 * M, :], o2_ps[:, :],
                                 AF.Copy)
            nc.sync.dma_start(oa_d[:, :], oa_sb[:, :])

    nc.compile()
    return nc


def host_prep(pred_logits, pred_masks, target_masks, target_classes,
              src_idx, tgt_idx, sample_idx, nch=NCH):
    """Compress/gather per-scene inputs and build per-core input maps."""
    sh = nch * 128
    npbf = mybir.dt.np(bf16)
    npf8 = mybir.dt.np(fp8)
    cls_w = np.ones(NCLS, np.float32)
    cls_w[0] = 0.0
    cls_w[-1] = EOS_COEF

    in_maps = []
    aux = {"tsum": [], "wsum": 0.0}
    zero_ce = np.zeros((Q, 2 * NCLS + 2), np.float32)
    for b in range(B):
        inv = np.argsort(tgt_idx[b])
        gidx = src_idx[b][inv]

        tc_full = np.full(Q, NUM_CLASSES, np.int64)
        tc_full[src_idx[b]] = target_classes[b][tgt_idx[b]]
        wq = cls_w[tc_full]
        aux["wsum"] += float(wq.sum())
        ce = np.zeros((Q, 2 * NCLS + 2), np.float32)
        ce[:, 0:NCLS] = pred_logits[b]
        ce[:, NCLS:2 * NCLS] = wq[:, None] * np.eye(NCLS, dtype=np.float32)[tc_full]
        ce[:, 2 * NCLS] = wq
        ce[:, 2 * NCLS + 1] = 1.0

        c_full = np.bincount(sample_idx[b], minlength=P)
        nz = np.flatnonzero(c_full)
        cnz = c_full[nz].astype(np.float64)
        ncomp = len(nz)
        assert ncomp <= 2 * sh, f"compressed points {ncomp} exceed capacity"

        assert cnz.max() <= 16, "counts exceed fp8-exact range"
        packed = np.zeros((2 * sh, 2 * M + 1), np.float32)
        packed[:ncomp, 0:M] = pred_masks[b][nz][:, gidx]
        ct = target_masks[b][:, nz].T * cnz[:, None]
        packed[:ncomp, M:2 * M] = ct
        packed[:ncomp, 2 * M] = cnz
        aux["tsum"].append(ct.sum(0))

        n0 = (ncomp + 1) // 2
        half0 = np.zeros((sh, 2 * M + 1), np.float32)
        half0[:n0] = packed[:n0]
        half1 = np.zeros((sh, 2 * M + 1), np.float32)
        half1[:ncomp - n0] = packed[n0:ncomp]
        for h, arr in enumerate((half0, half1)):
            a128 = np.ascontiguousarray(
                arr.reshape(nch, 128, 2 * M + 1).transpose(1, 0, 2))
            in_maps.append({
                "xin": np.ascontiguousarray(a128[:, :, 0:M]).astype(npbf),
                "tcx": np.ascontiguousarray(
                    a128[:, :, M:2 * M + 1]).astype(npf8),
                "ce": ce if h == 0 else zero_ce,
            })
    return in_maps, aux


def host_combine(results, aux):
    """results: list of 8 dicts with oa/oce. Returns [3] f32."""
    bce_total = 0.0
    dice_total = 0.0
    ce_num = 0.0
    idx = np.arange(M)
    for b in range(B):
        r0, r1 = results[2 * b], results[2 * b + 1]
        OA = r0["oa"].astype(np.float64) + r1["oa"]
        ce_num += float(r0["oa"][32, 0] - r0["oa"][33, 1])
        ce_num += float(r1["oa"][32, 0] - r1["oa"][33, 1])

        Tsum = aux["tsum"][b]
        X1 = OA[idx, idx].sum()
        ScS = Tsum - OA[64 + idx, idx]     # sum c*t*sigmoid
        Psum = NP - OA[64 + idx, M]        # sum c*sigmoid
        A = OA[64 + M + idx, M]            # sum c*softplus

        bce_total += A.sum() - X1
        dice_total += (1.0 - (2.0 * ScS + 1.0) / (Psum + Tsum + 1.0)).sum()

    num_masks = B * M
    loss_ce = ce_num / max(aux["wsum"], 1e-8)
    loss_mask = bce_total / NP / num_masks
    loss_dice = dice_total / num_masks
    return np.array([W_CE * loss_ce, W_DICE * loss_dice, W_MASK * loss_mask],
                    np.float32)


_NC_CACHE = {}


def kernel(pred_logits, pred_masks, target_masks, target_classes,
           src_idx, tgt_idx, sample_idx):
    from concourse.bass_utils import run_bass_kernel_spmd

    pred_logits = np.asarray(pred_logits, np.float32)
    pred_masks = np.asarray(pred_masks, np.float32)
    target_masks = np.asarray(target_masks, np.float32)
    target_classes = np.asarray(target_classes)
    src_idx = np.asarray(src_idx)
    tgt_idx = np.asarray(tgt_idx)
    sample_idx = np.asarray(sample_idx)

    if "nc" not in _NC_CACHE:
        _NC_CACHE["nc"] = build_nc()
    nc = _NC_CACHE["nc"]
    in_maps, aux = host_prep(
        pred_logits, pred_masks, target_masks, target_classes,
        src_idx, tgt_idx, sample_idx)
    res = run_bass_kernel_spmd(nc, in_maps, core_ids=list(range(8)))
    return host_combine(res.results, aux)
